# revision 25
# baseline (speedup 1.0000x reference)
"""Trainium2 Bass kernel for nn_AttentionCrossLayer.

Math: in the reference, softmax over a length-1 axis is exactly 1.0, so
attn == v and q/k/wq/wk are dead code. With x0 the (never-mutated) input,
each layer's gate xw_i = out_i @ cw_i is a fixed linear function of x0:
    xw_i = x0 @ u_i + c_i,   u_i = Wv_i @ (Wo_i @ cw_i),
                             c_i = (bv_i @ Wo_i + bo_i) @ cw_i
and the layer recurrence x += x0 * xw_i + cb_i telescopes to
    out[b, d] = x0[b, d] * (x0[b, :] @ usum + cprime) + cbsum[d]
with usum = sum_i u_i  [D], cprime = 1 + sum_i c_i, cbsum = sum_i cb_i [D].

The tiny weight contractions happen host-side in float64. The rel-err
gate is 2e-2, so x is staged to the device in bf16 and the output is
stored in bf16 (upcast to f32 on the host): this halves HBM traffic to
16.8MB/core. Quantization error ~0.2% RMS, measured rel err 2.5e-3.

The binding resource (measured): the DVE's fused multiply+row-reduce
(scalar_tensor_tensor with accum_out) runs at 1 elem/lane/cycle, so the
32 reduce windows of 1025 columns cost a fixed ~36.7us; HBM streaming
(~430GB/s/core peak) and the Scalar engine's 31 gate-multiplies
(~1.16us each) both fit underneath. The schedule therefore minimizes
DVE start latency and the post-compute tail.

Tiles: 128 rows (1 row/partition) for the first and last tile, 256
rows (2 consecutive rows/partition) for the 15 middle ones. The small
first tile gets the DVE working ~1.2us sooner; the small last tile
shortens the final dependency chain. Middle tiles put 2 rows per
partition so every DMA descriptor is a contiguous 4KB DRAM line (2KB
descriptors pay ~2x per-descriptor overhead on the 16 shared DMA
engines). All tiles stay SBUF-resident. Slot layout per partition
(bf16 elements, stride 2176 = 128B-aligned):
    [pad | c0 @ 63 | row0 (1024) | row1 (1024) | c1 @ 2112 | pad]
with 1.0 constants at 63 and 2112 (c1 unused by 128-row tiles). Reduce
windows are 1025 wide — each row plus one adjacent constant — and the
u operand is the broadcast row [cprime, usum, cprime] read at offset
0 resp. 1, so the reduce emits the finished gate t = x.usum + cprime
with no extra add op. The DMA destination stays 128B-aligned.

Schedule lessons baked in (from perfetto traces of prior versions):
- The u broadcast issues FIRST, from the sync engine, ahead of every x
  load: the DMA engines drain roughly in issue order, so issuing it
  later parks the DVE ~4us behind 5MB of queued loads.
- All loads issue back-to-back with no outstanding cap (everything is
  SBUF-resident; a cap only delays the last load and the whole tail).
- Warm-up ops: a dummy activation loads the Scalar engine's table
  (ACT_TABLE_LOAD, 1.3us) off the critical path, and a 1-element dummy
  STT runs on the DVE before its first real window.
- The last tile's pass 2 runs on the DVE (bf16 tensor_scalar, ~0.5us
  vs 1.16us on Scalar); Scalar multiplies tiles 0..15 and issues the
  trailing two stores on the by-then-idle HWDGE path; GpSimd (SWDGE)
  issues stores 0..14.
- GpSimd MUST await its SWDGE store completions before block end (the
  end-of-block drain resets SWDGE semaphore tracking; skipping the
  wait faults the device with NRT_EXEC_UNIT_UNRECOVERABLE). The
  HWDGE trailing stores are left in flight instead: the fixed ~7us
  end-of-block semaphore walk overlaps their drain.
- DVE instructions do NOT interlock RAW across the pipe: every read of
  an accum output or memset constant goes through a semaphore.

Sharding: data-parallel over batch across 8 cores, weights replicated,
no cross-device comms.
"""

import numpy as np

L, B, D, H, K = 3, 32768, 1024, 8, 64
N_CORES = 8
B_LOC = B // N_CORES  # 4096 rows per core
P = 128
XOFF = 64  # data offset inside a slot; element 63 is the chunk-0 constant
DPP = 2176  # slot stride in elements; 4352B = 128B aligned
W = D + 1  # reduce window width

# (row_start, rows_per_partition); first/last tiles are 128 rows
TILES = [(0, 1)] + [(128 + 256 * j, 2) for j in range(15)] + [(3968, 1)]
N_TILES = len(TILES)  # 17
LAST = N_TILES - 1

_cache = {}


def _build_program(zero_cb: bool):
    import concourse.bass as bass
    from concourse import mybir

    F32 = mybir.dt.float32
    BF16 = mybir.dt.bfloat16
    MUL = mybir.AluOpType.mult
    ADD = mybir.AluOpType.add

    nc = bass.Bass()
    x = nc.declare_dram_parameter("x", [B_LOC, D], BF16, isOutput=False)
    u = nc.declare_dram_parameter("u", [1, D + 2], BF16, isOutput=False)
    cb = nc.declare_dram_parameter("cb", [1, D], F32, isOutput=False)
    out = nc.declare_dram_parameter("out", [B_LOC, D], BF16, isOutput=True)

    u_bcast = bass.AP(tensor=u.ap().tensor, offset=0, ap=[[0, P], [1, D + 2]])
    cb_bcast = bass.AP(tensor=cb.ap().tensor, offset=0, ap=[[0, P], [1, D]])

    def dram_ap(t, i):
        # tile i as [128 partitions, R*D contiguous elements]
        rs, R = TILES[i]
        return bass.AP(tensor=t.ap().tensor, offset=rs * D, ap=[[R * D, P], [1, R * D]])

    # cumulative reduce-window counts for semaphore targets
    CW = [0]
    for _, R in TILES:
        CW.append(CW[-1] + R)
    TOTW = CW[-1]  # 32

    with (
        nc.sbuf_tensor([P, D + 2], BF16) as ub,  # [cprime, usum, cprime]
        nc.sbuf_tensor([P, D], F32) as cbb,
        nc.sbuf_tensor([P, N_TILES, DPP], BF16) as xt,
        # throwaway STT main outs; one slot per window so no WAW ordering
        # is needed (the 8-deep DVE pipe would otherwise race)
        nc.sbuf_tensor([P, TOTW, W], BF16) as oscr,
        nc.sbuf_tensor([P, TOTW], F32) as tsc,  # finished gates per window
        nc.sbuf_tensor([P, 2], BF16) as warm,  # warm-up scratch
        nc.sbuf_tensor([P, 1], F32) as warmt,
        nc.semaphore("us") as us,
        nc.semaphore("cm") as cm,    # STT accum writebacks retired (DVE)
        nc.semaphore("cm2") as cm2,  # row-chunks scaled (Scalar / DVE)
        nc.semaphore("cm3") as cm3,  # last tile scaled (DVE)
        nc.semaphore("st") as st,    # SWDGE store DMAs retired
        nc.semaphore("st2") as st2,  # HWDGE store DMAs retired (unwaited)
        nc.Block() as block,
    ):
        lds = [nc.alloc_semaphore(f"ld{i}") for i in range(N_TILES)]

        @block.sync
        def _(sync):
            # broadcast first: queues drain in rough issue order and the
            # DVE can't start until u lands
            sync.dma_start(out=ub[:, :], in_=u_bcast).then_inc(us, 16)
            if not zero_cb:
                sync.dma_start(out=cbb[:, :], in_=cb_bcast).then_inc(us, 16)
            for i in range(N_TILES):
                rd = TILES[i][1] * D
                sync.dma_start(
                    out=xt[:, i, XOFF : XOFF + rd], in_=dram_ap(x, i)
                ).then_inc(lds[i], 16)

        @block.vector
        def _(vector):
            # 1.0 constants adjacent to each reduce window; they ride the
            # cm chain (DVE has no same-engine RAW interlock)
            nc.vector.memset(xt[:, :, XOFF - 1 : XOFF], 1.0).then_inc(cm, 1)
            nc.vector.memset(
                xt[:, :, XOFF + FREE2 : XOFF + FREE2 + 1], 1.0
            ).then_inc(cm, 1)
            vector.wait_ge(cm, 2)
            # tiny dummy STT: absorbs the DVE's first-dispatch latency
            # while the broadcast and first load are still in flight
            nc.vector.scalar_tensor_tensor(
                out=warm[:, 0:1],
                in0=xt[:, 0, XOFF - 1 : XOFF],
                scalar=1.0,
                in1=xt[:, 1, XOFF - 1 : XOFF],
                op0=MUL,
                op1=MUL,
                accum_out=warmt[:, :],
            )
            vector.wait_ge(us, 16 if zero_cb else 32)
            for i in range(N_TILES):
                rs, R = TILES[i]
                vector.wait_ge(lds[i], 16)
                for r in range(R):
                    # oscr = win * u'; tsc[w] = sum = x_r . usum + cprime
                    w = CW[i] + r
                    nc.vector.scalar_tensor_tensor(
                        out=oscr[:, w, :],
                        in0=xt[:, i, XOFF - 1 + r * W : XOFF - 1 + (r + 1) * W],
                        scalar=1.0,
                        in1=ub[:, r : r + W],
                        op0=MUL,
                        op1=MUL,
                        accum_out=tsc[:, w : w + 1],
                    ).then_inc(cm, 1)
                if not zero_cb:
                    # general path: x <- x*t + cbsum on the DVE
                    vector.wait_ge(cm, 2 + CW[i + 1])
                    for r in range(R):
                        nc.vector.scalar_tensor_tensor(
                            out=xt[:, i, XOFF + r * D : XOFF + (r + 1) * D],
                            in0=xt[:, i, XOFF + r * D : XOFF + (r + 1) * D],
                            scalar=tsc[:, CW[i] + r : CW[i] + r + 1],
                            in1=cbb[:, :],
                            op0=MUL,
                            op1=ADD,
                        ).then_inc(cm2, 1)
            if zero_cb:
                # last tile's pass 2 on the DVE: shortest tail chain
                vector.wait_ge(cm, 2 + TOTW)
                nc.vector.tensor_scalar_mul(
                    out=xt[:, LAST, XOFF : XOFF + D],
                    in0=xt[:, LAST, XOFF : XOFF + D],
                    scalar1=tsc[:, TOTW - 1 : TOTW],
                ).then_inc(cm3, 1)

        @block.scalar
        def _(scalar):
            if zero_cb:
                # load the activation table off the critical path; read a
                # cell the u-broadcast initialized, write dead scratch
                scalar.wait_ge(us, 16)
                nc.scalar.mul(out=warm[:, 1:2], in_=ub[:, 0:1], mul=1.0)
                # pass 2: x_r <- x_r * t_r (cbsum == 0), per-partition
                # f32 scale AP on the activation path; tiles 0..15
                for i in range(N_TILES - 1):
                    for r in range(TILES[i][1]):
                        scalar.wait_ge(cm, 2 + CW[i] + r + 1)
                        nc.scalar.mul(
                            out=xt[:, i, XOFF + r * D : XOFF + (r + 1) * D],
                            in_=xt[:, i, XOFF + r * D : XOFF + (r + 1) * D],
                            mul=tsc[:, CW[i] + r : CW[i] + r + 1],
                        ).then_inc(cm2, 1)
                # trailing stores on the idle HWDGE path; self-wait on cm2
                # makes the in-place muls retire before the DMA reads
                scalar.wait_ge(cm2, CW[N_TILES - 1])
                scalar.dma_start(
                    out=dram_ap(out, N_TILES - 2),
                    in_=xt[:, N_TILES - 2, XOFF : XOFF + TILES[N_TILES - 2][1] * D],
                ).then_inc(st2, 16)
                scalar.wait_ge(cm3, 1)
                scalar.dma_start(
                    out=dram_ap(out, LAST), in_=xt[:, LAST, XOFF : XOFF + D]
                ).then_inc(st2, 16)
                # NO wait on st2: HWDGE transfers may drain under the
                # end-of-block semaphore walk

        @block.gpsimd
        def _(gpsimd):
            n_sw = N_TILES - 2 if zero_cb else N_TILES
            for i in range(n_sw):
                gpsimd.wait_ge(cm2, CW[i + 1])
                gpsimd.dma_start(
                    out=dram_ap(out, i),
                    in_=xt[:, i, XOFF : XOFF + TILES[i][1] * D],
                ).then_inc(st, 16)
            # SWDGE transfers MUST be awaited before block end: the
            # barrier's drain resets SWDGE semaphore tracking, and doing
            # so with stores in flight faults the device
            gpsimd.wait_ge(st, 16 * n_sw)

    return nc


FREE2 = 2 * D  # c1 position offset (relative to XOFF)


def _precompute(wv, bv, wo, bo, cw, cb):
    """Host-side f64 contraction of the small per-layer weights."""
    usum = np.zeros(D, np.float64)
    cprime = 1.0
    for i in range(L):
        Wv = wv[i].reshape(D, H * K).astype(np.float64)
        Wo = wo[i].reshape(H * K, D).astype(np.float64)
        cwi = cw[i].reshape(D).astype(np.float64)
        wocw = Wo @ cwi
        usum += Wv @ wocw
        cprime += float(bv[i].reshape(H * K).astype(np.float64) @ wocw)
        cprime += float(bo[i].astype(np.float64) @ cwi)
    cbsum = cb.astype(np.float64).sum(axis=0)
    return usum.astype(np.float32), float(np.float32(cprime)), cbsum.astype(np.float32)


def _ensure_trace_hook_importable():
    # bass_utils unconditionally imports antenv.axon_hooks when the
    # BASS_TRACE env var is set; some images lack that module. A None
    # hook makes bass_utils skip tracing gracefully.
    try:
        import antenv.axon_hooks  # noqa: F401
    except ImportError:
        import sys
        import types

        mod = types.ModuleType("antenv.axon_hooks")
        mod.get_axon_ntff_profile_hook = lambda: None
        mod.set_axon_ntff_profile_hook = lambda hook: None
        sys.modules["antenv.axon_hooks"] = mod


def kernel(x, wq, bq, wk, bk, wv, bv, wo, bo, cw, cb):
    import ml_dtypes

    from concourse.bass_utils import run_bass_kernel_spmd

    _ensure_trace_hook_importable()

    bf16 = np.dtype(ml_dtypes.bfloat16)
    x = np.ascontiguousarray(np.asarray(x, dtype=np.float32)).astype(bf16)
    usum, cprime, cbsum = _precompute(
        np.asarray(wv), np.asarray(bv), np.asarray(wo), np.asarray(bo),
        np.asarray(cw), np.asarray(cb),
    )
    zero_cb = not np.any(cbsum)

    if zero_cb not in _cache:
        _cache[zero_cb] = _build_program(zero_cb)
    nc = _cache[zero_cb]

    cp = np.float32(cprime)
    u2 = np.concatenate([[cp], usum, [cp]]).astype(bf16).reshape(1, D + 2)
    cb2 = cbsum.reshape(1, D)
    in_maps = [
        {"x": x[c * B_LOC : (c + 1) * B_LOC], "u": u2, "cb": cb2}
        for c in range(N_CORES)
    ]
    res = run_bass_kernel_spmd(nc, in_maps, list(range(N_CORES)))
    out16 = np.concatenate(
        [res.results[c]["out"] for c in range(N_CORES)], axis=0
    )
    return out16.astype(np.float32)


# revision 26
# speedup vs baseline: 1.0182x; 1.0182x over previous
"""Trainium2 Bass kernel for nn_AttentionCrossLayer. (v4b reconstruction)

out[b, d] = x0[b, d] * (x0[b, :] @ usum + cprime) + cbsum[d]; bf16 I/O.
See git-less history: v4b measured 56043ns.
"""

import numpy as np

L, B, D, H, K = 3, 32768, 1024, 8, 64
N_CORES = 8
B_LOC = B // N_CORES  # 4096 rows per core
P = 128
R = 2
N_TILES = B_LOC // (P * R)  # 16
FREE = R * D
XOFF = 64
C1 = XOFF + FREE
DPP = 2176
W = D + 1

_cache = {}


def _build_program(zero_cb: bool):
    import concourse.bass as bass
    from concourse import mybir

    F32 = mybir.dt.float32
    BF16 = mybir.dt.bfloat16
    MUL = mybir.AluOpType.mult
    ADD = mybir.AluOpType.add

    nc = bass.Bass()
    x = nc.declare_dram_parameter("x", [N_TILES * P, FREE], BF16, isOutput=False)
    u = nc.declare_dram_parameter("u", [1, D + 2], BF16, isOutput=False)
    cb = nc.declare_dram_parameter("cb", [1, D], F32, isOutput=False)
    out = nc.declare_dram_parameter("out", [N_TILES * P, FREE], BF16, isOutput=True)

    u_bcast = bass.AP(tensor=u.ap().tensor, offset=0, ap=[[0, P], [1, D + 2]])
    cb_bcast = bass.AP(tensor=cb.ap().tensor, offset=0, ap=[[0, P], [1, D]])

    LAST = N_TILES - 1

    with (
        nc.sbuf_tensor([P, D + 2], BF16) as ub,
        nc.sbuf_tensor([P, D], F32) as cbb,
        nc.sbuf_tensor([P, N_TILES, DPP], BF16) as xt,
        nc.sbuf_tensor([P, N_TILES, R, D + 1], BF16) as oscr,
        nc.sbuf_tensor([P, N_TILES, R], F32) as tsc,
        nc.sbuf_tensor([P, 1], BF16) as warm,
        nc.semaphore("us") as us,
        nc.semaphore("cm") as cm,
        nc.semaphore("cm2") as cm2,
        nc.semaphore("cm3") as cm3,
        nc.semaphore("st") as st,
        nc.semaphore("st2") as st2,
        nc.Block() as block,
    ):
        lds = [nc.alloc_semaphore(f"ld{i}") for i in range(N_TILES)]

        @block.sync
        def _(sync):
            sync.dma_start(out=ub[:, :], in_=u_bcast).then_inc(us, 16)
            if not zero_cb:
                sync.dma_start(out=cbb[:, :], in_=cb_bcast).then_inc(us, 16)
            for i in range(N_TILES):
                sync.dma_start(
                    out=xt[:, i, XOFF:C1], in_=x[i * P : (i + 1) * P, :]
                ).then_inc(lds[i], 16)

        @block.vector
        def _(vector):
            nc.vector.memset(xt[:, :, XOFF - 1 : XOFF], 1.0).then_inc(cm, 1)
            nc.vector.memset(xt[:, :, C1 : C1 + 1], 1.0).then_inc(cm, 1)
            vector.wait_ge(us, 16 if zero_cb else 32)
            vector.wait_ge(cm, 2)
            for i in range(N_TILES):
                vector.wait_ge(lds[i], 16)
                for r in range(R):
                    nc.vector.scalar_tensor_tensor(
                        out=oscr[:, i, r, :],
                        in0=xt[:, i, XOFF - 1 + r * W : XOFF - 1 + (r + 1) * W],
                        scalar=1.0,
                        in1=ub[:, r : r + W],
                        op0=MUL,
                        op1=MUL,
                        accum_out=tsc[:, i, r : r + 1],
                    ).then_inc(cm, 1)
                if not zero_cb:
                    vector.wait_ge(cm, 2 + R * (i + 1))
                    for r in range(R):
                        nc.vector.scalar_tensor_tensor(
                            out=xt[:, i, XOFF + r * D : XOFF + (r + 1) * D],
                            in0=xt[:, i, XOFF + r * D : XOFF + (r + 1) * D],
                            scalar=tsc[:, i, r : r + 1],
                            in1=cbb[:, :],
                            op0=MUL,
                            op1=ADD,
                        ).then_inc(cm2, 1)
            if zero_cb:
                vector.wait_ge(cm, 2 + R * N_TILES)
                for r in range(R):
                    nc.vector.tensor_scalar_mul(
                        out=xt[:, LAST, XOFF + r * D : XOFF + (r + 1) * D],
                        in0=xt[:, LAST, XOFF + r * D : XOFF + (r + 1) * D],
                        scalar1=tsc[:, LAST, r : r + 1],
                    ).then_inc(cm3, 1)

        @block.scalar
        def _(scalar):
            if zero_cb:
                scalar.wait_ge(us, 16)
                nc.scalar.mul(out=warm[:, :], in_=ub[:, 0:1], mul=1.0)
                for i in range(N_TILES - 1):
                    for r in range(R):
                        scalar.wait_ge(cm, 2 + R * i + r + 1)
                        nc.scalar.mul(
                            out=xt[:, i, XOFF + r * D : XOFF + (r + 1) * D],
                            in_=xt[:, i, XOFF + r * D : XOFF + (r + 1) * D],
                            mul=tsc[:, i, r : r + 1],
                        ).then_inc(cm2, 1)
                scalar.wait_ge(cm2, R * (N_TILES - 1))
                scalar.dma_start(
                    out=out[(N_TILES - 2) * P : (N_TILES - 1) * P, :],
                    in_=xt[:, N_TILES - 2, XOFF:C1],
                ).then_inc(st2, 16)
                scalar.wait_ge(cm3, R)
                scalar.dma_start(
                    out=out[LAST * P :, :], in_=xt[:, LAST, XOFF:C1]
                ).then_inc(st2, 16)
                scalar.wait_ge(st2, 32)

        @block.gpsimd
        def _(gpsimd):
            n_sw = N_TILES - 2 if zero_cb else N_TILES
            for i in range(n_sw):
                gpsimd.wait_ge(cm2, R * (i + 1))
                gpsimd.dma_start(
                    out=out[i * P : (i + 1) * P, :], in_=xt[:, i, XOFF:C1]
                ).then_inc(st, 16)
            gpsimd.wait_ge(st, 16 * n_sw)

    return nc


def _precompute(wv, bv, wo, bo, cw, cb):
    usum = np.zeros(D, np.float64)
    cprime = 1.0
    for i in range(L):
        Wv = wv[i].reshape(D, H * K).astype(np.float64)
        Wo = wo[i].reshape(H * K, D).astype(np.float64)
        cwi = cw[i].reshape(D).astype(np.float64)
        wocw = Wo @ cwi
        usum += Wv @ wocw
        cprime += float(bv[i].reshape(H * K).astype(np.float64) @ wocw)
        cprime += float(bo[i].astype(np.float64) @ cwi)
    cbsum = cb.astype(np.float64).sum(axis=0)
    return usum.astype(np.float32), float(np.float32(cprime)), cbsum.astype(np.float32)


def _ensure_trace_hook_importable():
    try:
        import antenv.axon_hooks  # noqa: F401
    except ImportError:
        import sys
        import types

        mod = types.ModuleType("antenv.axon_hooks")
        mod.get_axon_ntff_profile_hook = lambda: None
        mod.set_axon_ntff_profile_hook = lambda hook: None
        sys.modules["antenv.axon_hooks"] = mod


def kernel(x, wq, bq, wk, bk, wv, bv, wo, bo, cw, cb):
    import ml_dtypes

    from concourse.bass_utils import run_bass_kernel_spmd

    _ensure_trace_hook_importable()

    bf16 = np.dtype(ml_dtypes.bfloat16)
    x = np.ascontiguousarray(np.asarray(x, dtype=np.float32)).astype(bf16)
    usum, cprime, cbsum = _precompute(
        np.asarray(wv), np.asarray(bv), np.asarray(wo), np.asarray(bo),
        np.asarray(cw), np.asarray(cb),
    )
    zero_cb = not np.any(cbsum)

    if zero_cb not in _cache:
        _cache[zero_cb] = _build_program(zero_cb)
    nc = _cache[zero_cb]

    cp = np.float32(cprime)
    u2 = np.concatenate([[cp], usum, [cp]]).astype(bf16).reshape(1, D + 2)
    cb2 = cbsum.reshape(1, D)
    in_maps = [
        {
            "x": x[c * B_LOC : (c + 1) * B_LOC].reshape(N_TILES * P, FREE),
            "u": u2,
            "cb": cb2,
        }
        for c in range(N_CORES)
    ]
    res = run_bass_kernel_spmd(nc, in_maps, list(range(N_CORES)))
    out16 = np.concatenate(
        [res.results[c]["out"].reshape(B_LOC, D) for c in range(N_CORES)], axis=0
    )
    return out16.astype(np.float32)
